# revision 8
# baseline (speedup 1.0000x reference)
"""Trainium2 Bass kernel for nn_DualAxisAggAttn (dual-axis aggregation attention).

Computation (reference semantics, per batch image x[C=256, H=64, W=64]):
  stage W:  qkv = conv1x1(x) -> {q:[1], k:[C], v:[C]};  s = softmax_W(q)
            ctx[c,h] = sum_w k*s;  out = x + sigmoid(v) * ctx;  x_w = conv1x1(out)
  stage H:  same with softmax/reduction over H, applied to x_w.

Distribution: pure data-parallel over batch (16 images -> 2 per NeuronCore x 8).

Per-core kernel design (layout [C on 2x128 partitions, free = h*64+w] throughout):
  - 1x1 convs as PE matmuls in float32r (full PE rate, ~1.6e-4 rel err),
    stationary = transposed weight tiles, moving = x in 512-column chunks.
  - query row is replicated 128x in its own m-tile so exp(q) lands
    partition-broadcast for free.
  - ACT: exp, sigmoid, psum evictions.  DVE: k*E, grouped reductions
    (stage H reduces via transposed-view strided APs), combines.
    GPSIMD: gate*ctx with stride-0 broadcast APs.
  - biases: bq irrelevant (softmax shift-invariance); bk added to ctx;
    bv inside sigmoid bias; stage-W fus bias folded into stage-H
    (adjusted k/v biases + combine bias); stage-H fus bias on final evict.
"""

import os
import numpy as np
from contextlib import ExitStack

import concourse.bass as bass
import concourse.bacc as bacc
import concourse.tile as tile
import concourse.mybir as mybir
from concourse.bass_utils import run_bass_kernel_spmd

F32 = mybir.dt.float32
F32R = mybir.dt.float32r
AF = mybir.ActivationFunctionType
ALU = mybir.AluOpType
AX = mybir.AxisListType

B, C, H, W = 16, 256, 64, 64
HW = H * W
NCORES = 8
BPC = B // NCORES          # batches per core
KT = 2                     # k-tiles (C/128)
CH = 512                   # moving-chunk columns
NCH = HW // CH             # 8 chunks
GRP = CH // 64             # softmax groups per chunk (stage W)

_BUILD_CACHE = {}
LAST_RESULTS = None        # BassKernelResults of the most recent run (for test.py)


def _emit_stage(nc, pools, stage, src, stat, fus, bias, dst_evict):
    """Emit one attention stage for one batch.

    src:   sbuf tile [128, 2, HW] F32R (input, c-tiles on dim 1)
    stat:  sbuf [128, 2, 5, 128] F32R  (m-tiles: k0,k1,v0,v1,q)
    fus:   sbuf [128, 2, 2, 128] F32R
    bias:  dict with optional AP getters: 'bv' (list per ct), 'bk', 'bcomb'
    dst_evict(j, ps_f): consume fus psum [128,1024] for chunk j
    """
    (pbig, pchunk, pctx, pq, pk0, pk1, pv, pf) = pools
    axis_w = stage == "W"   # reduce over innermost (w) or outer (h)

    E = pbig.tile([128, HW], F32, tag="E")
    gate = pbig.tile([128, 2, HW], F32, tag="gate")
    t0 = pbig.tile([128, HW], F32, tag="t0")
    t1 = pbig.tile([128, HW], F32, tag="t1")

    bv = bias.get("bv")
    # ---- phase 1: qkv matmuls, evictions, k*E ----
    for j in range(NCH):
        sl = bass.ts(j, CH)
        ps_q = pq.tile([128, CH], F32, tag="q")
        ps_k0 = pk0.tile([128, CH], F32, tag="k0")
        ps_k1 = pk1.tile([128, CH], F32, tag="k1")
        ps_v = pv.tile([128, 2 * CH], F32, tag="v")
        for kt in range(KT):
            st, sp = kt == 0, kt == KT - 1
            rhs = src[:, kt, sl]
            nc.tensor.matmul(ps_k0[:], stat[:, kt, 0, :], rhs, start=st, stop=sp)
            nc.tensor.matmul(ps_k1[:], stat[:, kt, 1, :], rhs, start=st, stop=sp)
            nc.tensor.matmul(ps_v[:, 0:CH], stat[:, kt, 2, :], rhs, start=st, stop=sp)
            nc.tensor.matmul(ps_v[:, CH:], stat[:, kt, 3, :], rhs, start=st, stop=sp)
            nc.tensor.matmul(ps_q[:], stat[:, kt, 4, :], rhs, start=st, stop=sp)
        nc.scalar.activation(E[:, sl], ps_q[:], AF.Exp, bias=bias["zb"])
        if bv is None:
            nc.scalar.activation(
                gate[:, :, sl], ps_v[:].rearrange("p (c n) -> p c n", c=2),
                AF.Sigmoid, bias=bias["zb"],
            )
        else:
            nc.scalar.activation(gate[:, 0, sl], ps_v[:, 0:CH], AF.Sigmoid, bias=bv[0])
            nc.scalar.activation(gate[:, 1, sl], ps_v[:, CH:], AF.Sigmoid, bias=bv[1])
        nc.vector.tensor_tensor(t0[:, sl], ps_k0[:], E[:, sl], op=ALU.mult)
        nc.vector.tensor_tensor(t1[:, sl], ps_k1[:], E[:, sl], op=ALU.mult)

    # ---- phase 2: softmax stats + context ----
    def rview(flat):
        v = flat.rearrange("p (a r) -> p a r", r=64)
        return v if axis_w else v.transpose([0, 2, 1])

    S = pctx.tile([128, 64], F32, tag="S")
    nc.vector.tensor_reduce(S[:], rview(E[:]), axis=AX.X, op=ALU.add)
    R = pctx.tile([128, 64], F32, tag="R")
    nc.vector.reciprocal(R[:], S[:])
    ctxs = []
    for ct, t in enumerate((t0, t1)):
        cu = pctx.tile([128, 64], F32, tag=f"cu{ct}")
        nc.vector.tensor_reduce(cu[:], rview(t[:]), axis=AX.X, op=ALU.add)
        cn = pctx.tile([128, 64], F32, tag=f"cn{ct}")
        nc.vector.tensor_tensor(cn[:], cu[:], R[:], op=ALU.mult)
        if bias.get("bk") is not None:
            nc.vector.tensor_scalar_add(cn[:], cn[:], bias["bk"][ct])
        ctxs.append(cn)

    # ---- phase 3: gate*ctx, combine, fusion matmul ----
    bcomb = bias.get("bcomb")
    for j in range(NCH):
        sl = bass.ts(j, CH)
        outs = []
        for ct in range(2):
            if axis_w:
                cb = ctxs[ct][:, bass.ts(j, GRP)].unsqueeze(2).broadcast_to(
                    [128, GRP, 64]
                )
            else:
                cb = ctxs[ct][:].unsqueeze(1).broadcast_to([128, GRP, 64])
            g2 = pchunk.tile([128, GRP, 64], F32, tag=f"g2_{ct}")
            gv = gate[:, ct, sl].rearrange("p (a r) -> p a r", r=64)
            nc.gpsimd.tensor_tensor(g2[:], gv, cb, op=ALU.mult)
            o = pchunk.tile([128, CH], F32R, tag=f"out{ct}")
            g2f = g2[:].rearrange("p a r -> p (a r)")
            xin = src[:, ct, sl].bitcast(F32)
            if bcomb is None:
                nc.vector.tensor_tensor(o[:], xin, g2f, op=ALU.add)
            else:
                nc.vector.scalar_tensor_tensor(
                    o[:], xin, bcomb[ct], g2f, op0=ALU.add, op1=ALU.add
                )
            outs.append(o)
        ps_f = pf.tile([128, 2 * CH], F32, tag="f")
        for ct in range(2):
            st, sp = ct == 0, ct == 1
            nc.tensor.matmul(ps_f[:, 0:CH], fus[:, ct, 0, :], outs[ct][:], start=st, stop=sp)
            nc.tensor.matmul(ps_f[:, CH:], fus[:, ct, 1, :], outs[ct][:], start=st, stop=sp)
        dst_evict(j, ps_f)


def _build(flags):
    """flags: (bvW0, bkW0, bvH0, bkH0, bfW0, bfH0) booleans = 'is zero'."""
    bvW0, bkW0, bvH0, bkH0, bfW0, bfH0 = flags
    nc = bacc.Bacc(trn_type="TRN2", target_bir_lowering=False, debug=False)

    x_d = nc.dram_tensor("x", [BPC, C, HW], F32, kind="ExternalInput").ap()
    statW_d = nc.dram_tensor("statW", [128, KT, 5, 128], F32, kind="ExternalInput").ap()
    statH_d = nc.dram_tensor("statH", [128, KT, 5, 128], F32, kind="ExternalInput").ap()
    fusW_d = nc.dram_tensor("fusW", [128, KT, 2, 128], F32, kind="ExternalInput").ap()
    fusH_d = nc.dram_tensor("fusH", [128, KT, 2, 128], F32, kind="ExternalInput").ap()
    bias_d = nc.dram_tensor("biases", [6, 2, 128], F32, kind="ExternalInput").ap()
    y_d = nc.dram_tensor("y", [BPC, C, HW], F32, kind="ExternalOutput").ap()

    with tile.TileContext(nc) as tc, ExitStack() as ctx:
        wp = ctx.enter_context(tc.tile_pool(name="weights", bufs=1))
        xp = ctx.enter_context(tc.tile_pool(name="x", bufs=1))
        xwp = ctx.enter_context(tc.tile_pool(name="xw", bufs=1))
        pbig = ctx.enter_context(tc.tile_pool(name="big", bufs=1))
        pchunk = ctx.enter_context(tc.tile_pool(name="chunk", bufs=2))
        pctx = ctx.enter_context(tc.tile_pool(name="ctx", bufs=2))
        yp = ctx.enter_context(tc.tile_pool(name="yev", bufs=2))
        pq = ctx.enter_context(tc.tile_pool(name="psq", bufs=1, space="PSUM"))
        pk0 = ctx.enter_context(tc.tile_pool(name="psk0", bufs=1, space="PSUM"))
        pk1 = ctx.enter_context(tc.tile_pool(name="psk1", bufs=1, space="PSUM"))
        pv = ctx.enter_context(tc.tile_pool(name="psv", bufs=1, space="PSUM"))
        pf = ctx.enter_context(tc.tile_pool(name="psf", bufs=1, space="PSUM"))
        pools = (pbig, pchunk, pctx, pq, pk0, pk1, pv, pf)

        statW = wp.tile([128, KT, 5, 128], F32R, tag="statW")
        nc.gpsimd.dma_start(statW[:], statW_d[:])
        statH = wp.tile([128, KT, 5, 128], F32R, tag="statH")
        nc.gpsimd.dma_start(statH[:], statH_d[:])
        fusW = wp.tile([128, KT, 2, 128], F32R, tag="fusW")
        nc.gpsimd.dma_start(fusW[:], fusW_d[:])
        fusH = wp.tile([128, KT, 2, 128], F32R, tag="fusH")
        nc.gpsimd.dma_start(fusH[:], fusH_d[:])

        bias_sb = wp.tile([128, 6, 2], F32, tag="biases")
        nc.sync.dma_start(bias_sb[:], bias_d[:].transpose([2, 0, 1]))
        zb = wp.tile([128, 1], F32, tag="zb")
        nc.gpsimd.memset(zb[:], 0.0)

        def bap(i, ct):
            return bias_sb[:, i, ct].unsqueeze(1)

        biasW = {
            "bv": None if bvW0 else [bap(0, ct) for ct in range(2)],
            "bk": None if bkW0 else [bap(1, ct) for ct in range(2)],
            "bcomb": None,
            "zb": zb[:],
        }
        biasH = {
            "bv": None if bvH0 else [bap(2, ct) for ct in range(2)],
            "bk": None if bkH0 else [bap(3, ct) for ct in range(2)],
            "bcomb": None if bfW0 else [bap(4, ct) for ct in range(2)],
            "zb": zb[:],
        }

        for b in range(BPC):
            x_sb = xp.tile([128, KT, HW], F32R, tag="x")
            for kt in range(KT):
                half = HW // 2
                for piece in range(2):
                    psl = bass.ts(piece, half)
                    nc.gpsimd.dma_start(
                        x_sb[:, kt, psl], x_d[b, bass.ts(kt, 128), psl]
                    )

            xw_sb = xwp.tile([128, KT, HW], F32R, tag="xw")

            def evW(j, ps_f, xw_sb=xw_sb):
                nc.scalar.activation(
                    xw_sb[:, :, bass.ts(j, CH)],
                    ps_f[:].rearrange("p (c n) -> p c n", c=2),
                    AF.Copy,
                )

            _emit_stage(nc, pools, "W", x_sb[:], statW, fusW, biasW, evW)

            def evH(j, ps_f, b=b):
                y_t = yp.tile([128, 2, CH], F32, tag="y")
                if bfH0:
                    nc.scalar.activation(
                        y_t[:], ps_f[:].rearrange("p (c n) -> p c n", c=2), AF.Copy
                    )
                else:
                    for ct in range(2):
                        nc.scalar.activation(
                            y_t[:, ct, :], ps_f[:, bass.ts(ct, CH)],
                            AF.Identity, bias=bap(5, ct),
                        )
                nc.sync.dma_start(
                    y_d[b].rearrange("(c p) n -> p c n", p=128)[:, :, bass.ts(j, CH)],
                    y_t[:],
                )

            _emit_stage(nc, pools, "H", xw_sb[:], statH, fusH, biasH, evH)

    nc.compile()
    return nc


def _prep(qkv_w, qkv_b, fus_w):
    """stationary [128, kt, 5, 128] (m-tiles k0,k1,v0,v1,q) + fus [128, kt, 2, 128]."""
    wq = qkv_w[0]
    wk = qkv_w[1 : 1 + C]
    wv = qkv_w[1 + C :]
    stat = np.empty((128, KT, 5, 128), np.float32)
    fus = np.empty((128, KT, 2, 128), np.float32)
    for kt in range(KT):
        cs = slice(kt * 128, (kt + 1) * 128)
        stat[:, kt, 0, :] = wk[0:128, cs].T
        stat[:, kt, 1, :] = wk[128:256, cs].T
        stat[:, kt, 2, :] = wv[0:128, cs].T
        stat[:, kt, 3, :] = wv[128:256, cs].T
        stat[:, kt, 4, :] = np.repeat(wq[cs][:, None], 128, axis=1)
        fus[:, kt, 0, :] = fus_w[0:128, cs].T
        fus[:, kt, 1, :] = fus_w[128:256, cs].T
    return np.ascontiguousarray(stat), np.ascontiguousarray(fus)


def kernel(x, qkvW_w, qkvW_b, qkvH_w, qkvH_b, fusW_w, fusW_b, fusH_w, fusH_b):
    global LAST_RESULTS
    x = np.asarray(x, np.float32)
    qkvW_w = np.asarray(qkvW_w, np.float32)
    qkvW_b = np.asarray(qkvW_b, np.float32)
    qkvH_w = np.asarray(qkvH_w, np.float32)
    qkvH_b = np.asarray(qkvH_b, np.float32)
    fusW_w = np.asarray(fusW_w, np.float32)
    fusW_b = np.asarray(fusW_b, np.float32)
    fusH_w = np.asarray(fusH_w, np.float32)
    fusH_b = np.asarray(fusH_b, np.float32)

    statW, fusW = _prep(qkvW_w, qkvW_b, fusW_w)
    statH, fusH = _prep(qkvH_w, qkvH_b, fusH_w)

    bkW = qkvW_b[1 : 1 + C]
    bvW = qkvW_b[1 + C :]
    # stage-W fusion bias folds into stage H: k/v see x_w + bfW
    bkH = qkvH_b[1 : 1 + C] + qkvH_w[1 : 1 + C] @ fusW_b
    bvH = qkvH_b[1 + C :] + qkvH_w[1 + C :] @ fusW_b
    biases = np.stack(
        [bvW.reshape(2, 128),       # slot0: bvW
         bkW.reshape(2, 128),       # slot1: bkW
         bvH.reshape(2, 128),       # slot2: bvH eff
         bkH.reshape(2, 128),       # slot3: bkH eff
         fusW_b.reshape(2, 128),    # slot4: bfW (combine)
         fusH_b.reshape(2, 128)]    # slot5: bfH (final)
    ).astype(np.float32)

    flags = (
        not bvW.any(), not bkW.any(), not bvH.any(), not bkH.any(),
        not fusW_b.any(), not fusH_b.any(),
    )
    if flags not in _BUILD_CACHE:
        _BUILD_CACHE[flags] = _build(flags)
    nc = _BUILD_CACHE[flags]

    xr = np.ascontiguousarray(x.reshape(B, C, HW))
    in_maps = []
    for core in range(NCORES):
        in_maps.append({
            "x": xr[core * BPC : (core + 1) * BPC],
            "statW": statW, "statH": statH, "fusW": fusW, "fusH": fusH,
            "biases": biases,
        })

    res = run_bass_kernel_spmd(nc, in_maps, list(range(NCORES)))
    LAST_RESULTS = res
    y = np.concatenate([r["y"] for r in res.results], axis=0)
    return y.reshape(B, C, H, W)


# revision 11
# speedup vs baseline: 1.1110x; 1.1110x over previous
"""Trainium2 Bass kernel for nn_DualAxisAggAttn (dual-axis aggregation attention).

Reference semantics per batch image x[C=256, H=64, W=64], twice (W axis then H axis):
  qkv = conv1x1(x) -> {q:[1], k:[C], v:[C]};  s = softmax_axis(q)
  ctx[c,a] = sum_r k*s;  out = x + sigmoid(v) * ctx_bcast;  y = conv1x1(out)

Distribution: data-parallel over batch (16 images -> 2 per NeuronCore x 8 cores).

Key optimizations:
  - float32r matmuls (full PE rate, ~1.6e-4 rel err vs fp32).
  - key-path linearity: ctx = Wk @ (sum_r x*E) / S -- the key 1x1 conv moves
    AFTER the softmax-weighted reduction, shrinking it from N=4096 to N=64
    moving columns and removing the k psum tiles entirely.
  - query row replicated 128x in its m-tile -> exp(q) lands partition-broadcast.
  - sigmoid via tanh ((1+tanh(v/2))/2) so exp+tanh+copy live in ONE ACT table
    set (no ACT_TABLE_LOAD thrash); the +1 folds into a DVE scalar_tensor_tensor,
    the 0.5 into the softmax normalizer.
  - all reductions inner-contiguous: stage H writes u/E transposed (free at 1x).
  - bf16 E/u/gate/g2 intermediates; halving-add before reductions (bf16 2x mode).
  - engine split: DVE u-mult(ct0)/halves/reduces/g2; GP u-mult(ct1)/combines;
    ACT exp/tanh/evictions; PE matmuls.
"""

import numpy as np
from contextlib import ExitStack

import concourse.bass as bass
import concourse.bacc as bacc
import concourse.tile as tile
import concourse.mybir as mybir
from concourse.bass_utils import run_bass_kernel_spmd

F32 = mybir.dt.float32
F32R = mybir.dt.float32r
BF16 = mybir.dt.bfloat16
AF = mybir.ActivationFunctionType
ALU = mybir.AluOpType
AX = mybir.AxisListType

B, C, H, W = 16, 256, 64, 64
HW = H * W
NCORES = 8
BPC = B // NCORES
KT = 2
CH = 512
NCH = HW // CH
GRP = CH // 64

_BUILD_CACHE = {}
LAST_RESULTS = None


def _emit_stage(nc, pools, stage, src, stat, wk, fus, bias, dst_evict):
    """One attention stage for one batch.

    src:  [128, 2, HW] F32R input
    stat: [128, 2, 3, 128] F32R m-tiles (v0, v1, q-replicated)
    wk:   [128, 2, 2, 128] F32R key weights (ctx matmul)
    fus:  [128, 2, 2, 128] F32R fusion weights
    bias: dict: 'bv2' (halved v-bias APs or None), 'bk2' (halved k-bias APs or
          None), 'zb' zero [128,1]
    dst_evict(j, ps_f): consume fusion psum [128, 1024] for chunk j
    """
    (pbig, pchunk, pctx, pq, pv, pcx, pf) = pools
    axis_w = stage == "W"

    E = pbig.tile([128, HW], BF16, tag="E")
    gate = pbig.tile([128, 2, HW], BF16, tag="gate")
    u0 = pbig.tile([128, HW], BF16, tag="u0")
    u1 = pbig.tile([128, HW], BF16, tag="u1")

    def whv(flat, j):
        # chunk-j access [p, GRP, 64] of a wh-storage tile (element (h,w) at
        # flat w*64+h): strided view, free at 1x on DVE/ACT/GP.
        v3 = flat.rearrange("p (a r) -> p a r", r=64)
        return v3[:, :, bass.ts(j, GRP)].transpose([0, 2, 1])

    bv2 = bias.get("bv2")
    # ---- phase 1: q/v matmuls, E=exp(q), gate=tanh(v/2+bv/2), u = x*E ----
    for j in range(NCH):
        sl = bass.ts(j, CH)
        ps_q = pq.tile([128, CH], F32, tag="q")
        ps_v = pv.tile([128, 2 * CH], F32, tag="v")
        for kt in range(KT):
            st, sp = kt == 0, kt == KT - 1
            rhs = src[:, kt, sl]
            nc.tensor.matmul(ps_v[:, 0:CH], stat[:, kt, 0, :], rhs, start=st, stop=sp)
            nc.tensor.matmul(ps_v[:, CH:], stat[:, kt, 1, :], rhs, start=st, stop=sp)
            nc.tensor.matmul(ps_q[:], stat[:, kt, 2, :], rhs, start=st, stop=sp)
        if bv2 is None:
            nc.scalar.activation(
                gate[:, :, sl], ps_v[:].rearrange("p (c n) -> p c n", c=2),
                AF.Tanh, bias=bias["zb"], scale=0.5,
            )
        else:
            nc.scalar.activation(gate[:, 0, sl], ps_v[:, 0:CH], AF.Tanh, bias=bv2[0], scale=0.5)
            nc.scalar.activation(gate[:, 1, sl], ps_v[:, CH:], AF.Tanh, bias=bv2[1], scale=0.5)
        xin0 = src[:, 0, sl].bitcast(F32)
        xin1 = src[:, 1, sl].bitcast(F32)
        if axis_w:
            # natural storage: E/u flat chunks; reductions over w are inner.
            nc.scalar.activation(E[:, sl], ps_q[:], AF.Exp, bias=bias["zb"])
            nc.vector.tensor_tensor(u0[:, sl], xin0, E[:, sl], op=ALU.mult)
            nc.gpsimd.tensor_tensor(u1[:, sl], xin1, E[:, sl], op=ALU.mult)
        else:
            # wh storage: write E/u transposed so reductions over h are inner.
            q3 = ps_q[:].rearrange("p (a r) -> p a r", r=64)
            nc.scalar.activation(whv(E[:], j), q3, AF.Exp, bias=bias["zb"])
            x3 = [xin0.rearrange("p (a r) -> p a r", r=64),
                  xin1.rearrange("p (a r) -> p a r", r=64)]
            nc.vector.tensor_tensor(whv(u0[:], j), x3[0], whv(E[:], j), op=ALU.mult)
            nc.gpsimd.tensor_tensor(whv(u1[:], j), x3[1], whv(E[:], j), op=ALU.mult)

    # ---- phase 2: reductions (all inner-contiguous now), ctx matmul ----
    def halve_reduce(flat, tag):
        # [p, (a,64)] bf16 -> halving add (2x mode) -> reduce 32 -> [p, 64] f32
        v3 = flat.rearrange("p (a r) -> p a r", r=64)
        hv = pctx.tile([128, 64, 32], BF16, tag=f"hv_{tag}")
        nc.vector.tensor_tensor(hv[:], v3[:, :, 0:32], v3[:, :, 32:64], op=ALU.add)
        out = pctx.tile([128, 64], F32, tag=f"red_{tag}")
        nc.vector.tensor_reduce(out[:], hv[:], axis=AX.X, op=ALU.add)
        return out

    S = halve_reduce(E[:], "S")
    R = pctx.tile([128, 64], F32, tag="R")
    nc.vector.reciprocal(R[:], S[:])
    xen = []
    for ct, u in enumerate((u0, u1)):
        xe = halve_reduce(u[:], f"xe{ct}")
        xn = pctx.tile([128, 64], F32R, tag=f"xn{ct}")
        nc.vector.tensor_tensor(xn[:], xe[:], R[:], op=ALU.mult)
        xen.append(xn)
    ctxs = []
    for mt in range(2):
        ps_c = pcx.tile([128, 64], F32, tag=f"c{mt}")
        for ct in range(2):
            nc.tensor.matmul(ps_c[:], wk[:, ct, mt, :], xen[ct][:], start=ct == 0, stop=ct == 1)
        cn = pctx.tile([128, 64], F32, tag=f"cn{mt}")
        bk2 = bias.get("bk2")
        if bk2 is None:
            nc.vector.tensor_scalar_mul(cn[:], ps_c[:], 0.5)
        else:
            nc.vector.tensor_scalar(cn[:], ps_c[:], 0.5, bk2[mt], op0=ALU.mult, op1=ALU.add)
        ctxs.append(cn)

    # ---- phase 3: g2 = (gate+1)*ctx', out = x + g2, fusion matmul ----
    for j in range(NCH):
        sl = bass.ts(j, CH)
        outs = []
        for ct in range(2):
            if axis_w:
                cb = ctxs[ct][:, bass.ts(j, GRP)].unsqueeze(2).broadcast_to([128, GRP, 64])
            else:
                cb = ctxs[ct][:].unsqueeze(1).broadcast_to([128, GRP, 64])
            g2 = pchunk.tile([128, GRP, 64], BF16, tag=f"g2_{ct}")
            gv = gate[:, ct, sl].rearrange("p (a r) -> p a r", r=64)
            nc.vector.scalar_tensor_tensor(g2[:], gv, 1.0, cb, op0=ALU.add, op1=ALU.mult)
            o = pchunk.tile([128, CH], F32R, tag=f"out{ct}")
            nc.gpsimd.tensor_tensor(
                o[:], src[:, ct, sl].bitcast(F32),
                g2[:].rearrange("p a r -> p (a r)"), op=ALU.add,
            )
            outs.append(o)
        ps_f = pf.tile([128, 2 * CH], F32, tag="f")
        for ct in range(2):
            st, sp = ct == 0, ct == 1
            nc.tensor.matmul(ps_f[:, 0:CH], fus[:, ct, 0, :], outs[ct][:], start=st, stop=sp)
            nc.tensor.matmul(ps_f[:, CH:], fus[:, ct, 1, :], outs[ct][:], start=st, stop=sp)
        dst_evict(j, ps_f)


def _build(flags):
    bvW0, bkW0, bvH0, bkH0, bfW0, bfH0 = flags
    nc = bacc.Bacc(trn_type="TRN2", target_bir_lowering=False, debug=False)

    x_d = nc.dram_tensor("x", [BPC, C, HW], F32, kind="ExternalInput").ap()
    statW_d = nc.dram_tensor("statW", [128, KT, 3, 128], F32, kind="ExternalInput").ap()
    statH_d = nc.dram_tensor("statH", [128, KT, 3, 128], F32, kind="ExternalInput").ap()
    wkW_d = nc.dram_tensor("wkW", [128, KT, 2, 128], F32, kind="ExternalInput").ap()
    wkH_d = nc.dram_tensor("wkH", [128, KT, 2, 128], F32, kind="ExternalInput").ap()
    fusW_d = nc.dram_tensor("fusW", [128, KT, 2, 128], F32, kind="ExternalInput").ap()
    fusH_d = nc.dram_tensor("fusH", [128, KT, 2, 128], F32, kind="ExternalInput").ap()
    bias_d = nc.dram_tensor("biases", [6, 2, 128], F32, kind="ExternalInput").ap()
    y_d = nc.dram_tensor("y", [BPC, C, HW], F32, kind="ExternalOutput").ap()

    with tile.TileContext(nc) as tc, ExitStack() as ctx:
        wp = ctx.enter_context(tc.tile_pool(name="weights", bufs=1))
        xp = ctx.enter_context(tc.tile_pool(name="x", bufs=2))
        xwp = ctx.enter_context(tc.tile_pool(name="xw", bufs=1))
        pbig = ctx.enter_context(tc.tile_pool(name="big", bufs=1))
        pchunk = ctx.enter_context(tc.tile_pool(name="chunk", bufs=3))
        pctx = ctx.enter_context(tc.tile_pool(name="ctx", bufs=2))
        yp = ctx.enter_context(tc.tile_pool(name="yev", bufs=2))
        pq = ctx.enter_context(tc.tile_pool(name="psq", bufs=1, space="PSUM"))
        pv = ctx.enter_context(tc.tile_pool(name="psv", bufs=1, space="PSUM"))
        pcx = ctx.enter_context(tc.tile_pool(name="pscx", bufs=1, space="PSUM"))
        pf = ctx.enter_context(tc.tile_pool(name="psf", bufs=1, space="PSUM"))
        pools = (pbig, pchunk, pctx, pq, pv, pcx, pf)

        def wload(name, dram, shape):
            t = wp.tile(shape, F32R, tag=name)
            nc.gpsimd.dma_start(t[:], dram[:])
            return t

        statW = wload("statW", statW_d, [128, KT, 3, 128])
        statH = wload("statH", statH_d, [128, KT, 3, 128])
        wkW = wload("wkW", wkW_d, [128, KT, 2, 128])
        wkH = wload("wkH", wkH_d, [128, KT, 2, 128])
        fusW = wload("fusW", fusW_d, [128, KT, 2, 128])
        fusH = wload("fusH", fusH_d, [128, KT, 2, 128])

        bias_sb = wp.tile([128, 6, 2], F32, tag="biases")
        nc.sync.dma_start(bias_sb[:], bias_d[:].transpose([2, 0, 1]))
        zb = wp.tile([128, 1], F32, tag="zb")
        nc.gpsimd.memset(zb[:], 0.0)

        def bap(i, ct):
            return bias_sb[:, i, ct].unsqueeze(1)

        biasW = {
            "bv2": None if bvW0 else [bap(0, ct) for ct in range(2)],
            "bk2": None if bkW0 else [bap(1, ct) for ct in range(2)],
            "zb": zb[:],
        }
        biasH = {
            "bv2": None if bvH0 else [bap(2, ct) for ct in range(2)],
            "bk2": None if bkH0 else [bap(3, ct) for ct in range(2)],
            "zb": zb[:],
        }

        for b in range(BPC):
            x_sb = xp.tile([128, KT, HW], F32R, tag="x")
            for kt in range(KT):
                half = HW // 2
                for piece in range(2):
                    psl = bass.ts(piece, half)
                    nc.gpsimd.dma_start(x_sb[:, kt, psl], x_d[b, bass.ts(kt, 128), psl])

            xw_sb = xwp.tile([128, KT, HW], F32R, tag="xw")

            def evW(j, ps_f, xw_sb=xw_sb):
                pv2 = ps_f[:].rearrange("p (c n) -> p c n", c=2)
                if bfW0:
                    nc.scalar.activation(xw_sb[:, :, bass.ts(j, CH)], pv2, AF.Copy)
                else:
                    for ct in range(2):
                        nc.scalar.activation(
                            xw_sb[:, ct, bass.ts(j, CH)], ps_f[:, bass.ts(ct, CH)],
                            AF.Identity, bias=bap(4, ct),
                        )

            _emit_stage(nc, pools, "W", x_sb[:], statW, wkW, fusW, biasW, evW)

            def evH(j, ps_f, b=b):
                y_t = yp.tile([128, 2, CH], F32, tag="y")
                if bfH0:
                    nc.scalar.activation(y_t[:], ps_f[:].rearrange("p (c n) -> p c n", c=2), AF.Copy)
                else:
                    for ct in range(2):
                        nc.scalar.activation(
                            y_t[:, ct, :], ps_f[:, bass.ts(ct, CH)],
                            AF.Identity, bias=bap(5, ct),
                        )
                nc.sync.dma_start(
                    y_d[b].rearrange("(c p) n -> p c n", p=128)[:, :, bass.ts(j, CH)],
                    y_t[:],
                )

            _emit_stage(nc, pools, "H", xw_sb[:], statH, wkH, fusH, biasH, evH)

    nc.compile()
    return nc


def _prep(qkv_w, fus_w):
    wq = qkv_w[0]
    wk = qkv_w[1 : 1 + C]
    wv = qkv_w[1 + C :]
    stat = np.empty((128, KT, 3, 128), np.float32)
    wkt = np.empty((128, KT, 2, 128), np.float32)
    fus = np.empty((128, KT, 2, 128), np.float32)
    for kt in range(KT):
        cs = slice(kt * 128, (kt + 1) * 128)
        stat[:, kt, 0, :] = wv[0:128, cs].T
        stat[:, kt, 1, :] = wv[128:256, cs].T
        stat[:, kt, 2, :] = np.repeat(wq[cs][:, None], 128, axis=1)
        wkt[:, kt, 0, :] = wk[0:128, cs].T
        wkt[:, kt, 1, :] = wk[128:256, cs].T
        fus[:, kt, 0, :] = fus_w[0:128, cs].T
        fus[:, kt, 1, :] = fus_w[128:256, cs].T
    return (np.ascontiguousarray(stat), np.ascontiguousarray(wkt),
            np.ascontiguousarray(fus))


def kernel(x, qkvW_w, qkvW_b, qkvH_w, qkvH_b, fusW_w, fusW_b, fusH_w, fusH_b):
    global LAST_RESULTS
    x = np.asarray(x, np.float32)
    qkvW_w = np.asarray(qkvW_w, np.float32)
    qkvW_b = np.asarray(qkvW_b, np.float32)
    qkvH_w = np.asarray(qkvH_w, np.float32)
    qkvH_b = np.asarray(qkvH_b, np.float32)
    fusW_w = np.asarray(fusW_w, np.float32)
    fusW_b = np.asarray(fusW_b, np.float32)
    fusH_w = np.asarray(fusH_w, np.float32)
    fusH_b = np.asarray(fusH_b, np.float32)

    statW, wkW, fusW = _prep(qkvW_w, fusW_w)
    statH, wkH, fusH = _prep(qkvH_w, fusH_w)

    bkW = qkvW_b[1 : 1 + C]
    bvW = qkvW_b[1 + C :]
    bkH = qkvH_b[1 : 1 + C]
    bvH = qkvH_b[1 + C :]
    biases = np.stack(
        [0.5 * bvW.reshape(2, 128),   # slot0: bvW/2 (tanh bias)
         0.5 * bkW.reshape(2, 128),   # slot1: bkW/2 (ctx add)
         0.5 * bvH.reshape(2, 128),   # slot2: bvH/2
         0.5 * bkH.reshape(2, 128),   # slot3: bkH/2
         fusW_b.reshape(2, 128),      # slot4: bfW (xw evict bias)
         fusH_b.reshape(2, 128)]      # slot5: bfH (y evict bias)
    ).astype(np.float32)

    flags = (
        not bvW.any(), not bkW.any(), not bvH.any(), not bkH.any(),
        not fusW_b.any(), not fusH_b.any(),
    )
    if flags not in _BUILD_CACHE:
        _BUILD_CACHE[flags] = _build(flags)
    nc = _BUILD_CACHE[flags]

    xr = np.ascontiguousarray(x.reshape(B, C, HW))
    in_maps = []
    for core in range(NCORES):
        in_maps.append({
            "x": xr[core * BPC : (core + 1) * BPC],
            "statW": statW, "statH": statH,
            "wkW": wkW, "wkH": wkH, "fusW": fusW, "fusH": fusH,
            "biases": biases,
        })

    res = run_bass_kernel_spmd(nc, in_maps, list(range(NCORES)))
    LAST_RESULTS = res
    y = np.concatenate([r["y"] for r in res.results], axis=0)
    return y.reshape(B, C, H, W)
